# revision 1
# baseline (speedup 1.0000x reference)
import sys
from contextlib import ExitStack

import numpy as np
import ml_dtypes

sys.path.insert(0, "/opt/trn_rl_repo")

try:
    import jax
    jax.config.update("jax_compilation_cache_dir", "/tmp/jax_cc_cache")
    jax.config.update("jax_persistent_cache_min_compile_time_secs", 0.0)
    jax.config.update("jax_persistent_cache_min_entry_size_bytes", 0)
except Exception:
    pass

import concourse.bass as bass
import concourse.tile as tile
from concourse import bacc, mybir
from concourse.bass_utils import run_bass_kernel_spmd

B, H, W, CH = 4, 80, 80, 256
NCLS, DIM = 22, 256
ROWS = 40            # rows per core
NPIX = ROWS * W      # 3200 output pixels per core
NT = (ROWS + 2) * W + 2   # 3362 strip positions (1 halo row each side + 1 elem pad)
NTILE = NPIX // 128  # 25 output tiles of 128 pixels
SELW = 9 * 128       # per-tile selp row width (k-major, pixel minor)
# merged-input column layout (bf16): [x strip | sel | full weights]
XC = 2 * NT               # 6724 cols of x data
SC = NTILE * SELW // 128  # 225 cols of sel data
WC = 18 * DIM             # 4608 cols of full weights
TOTC = XC + SC + WC       # 11557
F32 = mybir.dt.float32
F16 = mybir.dt.float16
BF16 = mybir.dt.bfloat16
BF16NP = ml_dtypes.bfloat16


def _build_nc():
    nc = bacc.Bacc("TRN2", target_bir_lowering=False, debug=False,
                   enable_asserts=True, num_devices=8)
    # single merged bf16 input: x strip cols [0:XC), sel cols [XC:XC+SC),
    # full replicated weights cols [XC+SC:TOTC)
    inx_d = nc.dram_tensor("inx", [128, TOTC], BF16, kind="ExternalInput").ap()
    out_d = nc.dram_tensor("out", [NPIX, DIM], F16, kind="ExternalOutput").ap()

    with tile.TileContext(nc) as tc, ExitStack() as ctx:
        xp = ctx.enter_context(tc.tile_pool(name="xp", bufs=1))
        wp = ctx.enter_context(tc.tile_pool(name="wp", bufs=1))
        stp = ctx.enter_context(tc.tile_pool(name="stp", bufs=1))
        Sp = ctx.enter_context(tc.tile_pool(name="Sp", bufs=3))
        xtsp = ctx.enter_context(tc.tile_pool(name="xtsp", bufs=3))
        outp = ctx.enter_context(tc.tile_pool(name="outp", bufs=3))
        zp = ctx.enter_context(tc.tile_pool(name="zp", bufs=6, space="PSUM"))

        xt = xp.tile([128, XC], BF16)
        wt = wp.tile([128, WC], BF16)
        selt = stp.tile([1, NTILE * SELW], BF16)

        # sel in 4 row-aligned chunks so the first broadcasts start early
        for c in range(4):
            nc.sync.dma_start(selt[0:1, c * 7200:(c + 1) * 7200],
                              inx_d[c * 32:(c + 1) * 32, XC:XC + SC])
        # x chunk 0 first (tile 0's multiply needs it), then weights, then rest
        bnds = [0, 850, 1700, 2550, NT]
        for h in range(2):
            nc.sync.dma_start(xt[:, h * NT:h * NT + bnds[1]],
                              inx_d[:, h * NT:h * NT + bnds[1]])
        for k in range(9):
            nc.sync.dma_start(wt[:, k * 512:(k + 1) * 512],
                              inx_d[:, XC + SC + k * 512:XC + SC + (k + 1) * 512])
        for ci in range(1, 4):
            for h in range(2):
                a, b = h * NT + bnds[ci], h * NT + bnds[ci + 1]
                nc.sync.dma_start(xt[:, a:b], inx_d[:, a:b])

        for j in range(NTILE):
            S = Sp.tile([128, SELW], BF16)
            nc.gpsimd.partition_broadcast(
                S[:], selt[0:1, j * SELW:(j + 1) * SELW])
            xts = xtsp.tile([128, 2 * SELW], BF16)
            xb = xt[:, 0:1]
            pstep = xb.ap[0][0]
            for h in range(2):
                g = bass.AP(xb.tensor, xb.offset + h * NT + j * 128,
                            [[pstep, 128], [80, 3], [1, 3], [1, 128]])
                nc.vector.tensor_mul(xts[:, h * SELW:(h + 1) * SELW], g, S[:])
            z = zp.tile([128, DIM], F32)
            for k in range(9):
                for h in range(2):
                    nc.tensor.matmul(
                        z[:],
                        xts[:, h * SELW + k * 128:h * SELW + (k + 1) * 128],
                        wt[:, (2 * k + h) * DIM:(2 * k + h + 1) * DIM],
                        start=(k == 0 and h == 0), stop=(k == 8 and h == 1))
            outt = outp.tile([128, DIM], F16)
            nc.scalar.copy(outt[:], z[:])
            nc.sync.dma_start(out_d[j * 128:(j + 1) * 128, :], outt[:])
    nc.compile()
    return nc


_NC_CACHE = None


def _get_nc():
    global _NC_CACHE
    if _NC_CACHE is None:
        _NC_CACHE = _build_nc()
    return _NC_CACHE


def _prep_core(x, seg_mask, core):
    b, r0 = core // 2, 40 * (core % 2)
    xp = np.pad(x[b], ((1, 1), (0, 0), (0, 0)))        # [82,80,256]
    strip = xp[r0:r0 + 42].reshape(42 * W, CH)
    sp = np.zeros((NT, CH), np.float32)
    sp[1:1 + 42 * W] = strip
    spT = sp.T
    xt = np.ascontiguousarray(
        np.concatenate([spT[:128], spT[128:]], axis=1)).astype(BF16NP)

    pads = np.pad(seg_mask[b], ((1, 1), (1, 1), (0, 0)))  # [82,82,22]
    mc = seg_mask[b][r0:r0 + 40]                          # [40,80,22]
    smax = mc.max(-1, keepdims=True)
    eq = (mc == smax).astype(np.float32)
    sel = np.empty((40, 80, 9), np.float32)
    for k in range(9):
        di, dj = k // 3 - 1, k % 3 - 1
        sel[..., k] = (eq * pads[r0 + 1 + di:r0 + 41 + di,
                                 1 + dj:81 + dj]).sum(-1)
    cnt = (sel != 0).astype(np.float32).sum(-1, keepdims=True)
    selp = sel * (9.0 / np.maximum(cnt, 1.0))
    # [NTILE, 9, 128]: k-major, pixel-in-tile minor
    selt = np.ascontiguousarray(
        selp.reshape(NTILE, 128, 9).transpose(0, 2, 1)
    ).astype(BF16NP).reshape(128, SC)
    return xt, selt


def _prep_in_maps(x, seg_mask, conv_w):
    w9 = conv_w.reshape(CH, 9, DIM)
    # [128, 9, 2, 256]: per k, both ch halves adjacent
    wt = np.ascontiguousarray(
        np.stack([w9[:128], w9[128:]], axis=2).reshape(128, WC)
    ).astype(BF16NP)

    in_maps = []
    for core in range(8):
        xt, selt = _prep_core(x, seg_mask, core)
        inx = np.empty((128, TOTC), BF16NP)
        inx[:, :XC] = xt
        inx[:, XC:XC + SC] = selt
        inx[:, XC + SC:] = wt
        in_maps.append({"inx": inx})
    return in_maps


def kernel(x, seg_mask, conv_w):
    x = np.asarray(x, np.float32)
    seg_mask = np.asarray(seg_mask, np.float32)
    conv_w = np.asarray(conv_w, np.float32)

    in_maps = _prep_in_maps(x, seg_mask, conv_w)
    nc = _get_nc()
    res = run_bass_kernel_spmd(nc, in_maps, core_ids=list(range(8)))

    out = np.empty((B, H, W, DIM), np.float32)
    for core in range(8):
        b, r0 = core // 2, 40 * (core % 2)
        out[b, r0:r0 + 40] = res.results[core]["out"].astype(
            np.float32).reshape(ROWS, W, DIM)
    return out



# revision 2
# speedup vs baseline: 1.2302x; 1.2302x over previous
import sys
from contextlib import ExitStack

import numpy as np
import ml_dtypes

sys.path.insert(0, "/opt/trn_rl_repo")

try:
    import jax
    jax.config.update("jax_compilation_cache_dir", "/tmp/jax_cc_cache")
    jax.config.update("jax_persistent_cache_min_compile_time_secs", 0.0)
    jax.config.update("jax_persistent_cache_min_entry_size_bytes", 0)
except Exception:
    pass

import concourse.bass as bass  # noqa: F401
import concourse.tile as tile
from concourse import bacc, mybir
from concourse.bass_utils import run_bass_kernel_spmd

B, H, W, CH = 4, 80, 80, 256
NCLS, DIM = 22, 256
ROWS = 40                 # rows per core
NPIX = ROWS * W           # 3200 output pixels per core
NT = (ROWS + 2) * W + 2   # 3362 strip positions (halo rows + end pad)
G = 512                   # pixel chunk
CHUNKS = [(g, min(G, NPIX - g)) for g in range(0, NPIX, G)]  # 6x512 + 128
SELC = 9 * NPIX           # 28800 sel columns (chunk-major, k-major, pixel)
WC = 36 * 128             # 4608 weight cols: block (k,h,dh) of [128c x 128d]
F32 = mybir.dt.float32
F16 = mybir.dt.float16
BF16 = mybir.dt.bfloat16
BF16NP = ml_dtypes.bfloat16


def _build_nc():
    nc = bacc.Bacc("TRN2", target_bir_lowering=False, debug=False,
                   enable_asserts=True, num_devices=8)
    xa_d = nc.dram_tensor("xa", [128, 2 * NT], BF16, kind="ExternalInput").ap()
    xb_d = nc.dram_tensor("xb", [128, 2 * NT], BF16, kind="ExternalInput").ap()
    sel_d = nc.dram_tensor("selb", [128, SELC], BF16, kind="ExternalInput").ap()
    wt_d = nc.dram_tensor("wt", [128, WC], BF16, kind="ExternalInput").ap()
    out_d = nc.dram_tensor("out", [2 * 128, NPIX], F16, kind="ExternalOutput").ap()

    with tile.TileContext(nc) as tc, ExitStack() as ctx:
        xp = ctx.enter_context(tc.tile_pool(name="xp", bufs=1))
        Sp = ctx.enter_context(tc.tile_pool(name="Sp", bufs=3))
        xgp = ctx.enter_context(tc.tile_pool(name="xgp", bufs=3))
        outp = ctx.enter_context(tc.tile_pool(name="outp", bufs=4))
        zp = ctx.enter_context(tc.tile_pool(name="zp", bufs=4, space="PSUM"))

        xa_t = xp.tile([128, 2 * NT], BF16)
        xb_t = xp.tile([128, 2 * NT], BF16)
        wt_t = xp.tile([128, WC], BF16)

        # x pieces: chunk0-critical region first, then the rest
        XP = [(0, 848), (848, 1024), (1872, NT - 1872)]
        for t, d in ((xa_t, xa_d), (xb_t, xb_d)):
            a, L = XP[0]
            for h in range(2):
                nc.sync.dma_start(t[:, h * NT + a:h * NT + a + L],
                                  d[:, h * NT + a:h * NT + a + L])

        sel_off = [0]
        for _, g in CHUNKS:
            sel_off.append(sel_off[-1] + 9 * g)

        for ci, (g0, g) in enumerate(CHUNKS):
            S = Sp.tile([128, 9 * G], BF16)
            if ci == 0:
                # fine-grained so gating of k=0 starts asap
                for p in range(3):
                    nc.sync.dma_start(
                        S[:, p * 3 * g:(p + 1) * 3 * g],
                        sel_d[:, sel_off[0] + p * 3 * g:sel_off[0] + (p + 1) * 3 * g])
                # weights + remaining x pieces issue behind chunk0 criticals
                nc.sync.dma_start(wt_t[:, :WC // 2], wt_d[:, :WC // 2])
                nc.sync.dma_start(wt_t[:, WC // 2:], wt_d[:, WC // 2:])
                for t, d in ((xa_t, xa_d), (xb_t, xb_d)):
                    for a, L in XP[1:]:
                        for h in range(2):
                            nc.sync.dma_start(t[:, h * NT + a:h * NT + a + L],
                                              d[:, h * NT + a:h * NT + a + L])
            else:
                nc.sync.dma_start(S[:, :9 * g],
                                  sel_d[:, sel_off[ci]:sel_off[ci] + 9 * g])

            xg = xgp.tile([128, 18 * G], BF16)
            for k in range(9):
                di, dj = k // 3, k % 3
                for h in range(2):
                    if dj == 1:
                        src = xb_t[:, h * NT + g0 + 80 * di:
                                   h * NT + g0 + 80 * di + g]
                    else:
                        src = xa_t[:, h * NT + g0 + 80 * di + dj:
                                   h * NT + g0 + 80 * di + dj + g]
                    nc.vector.tensor_mul(xg[:, (2 * k + h) * g:(2 * k + h + 1) * g],
                                         src, S[:, k * g:(k + 1) * g])

            for dh in range(2):
                z = zp.tile([128, G], F32)
                for k in range(9):
                    for h in range(2):
                        blk = (k * 2 + h) * 2 + dh
                        nc.tensor.matmul(
                            z[:, :g],
                            wt_t[:, blk * 128:(blk + 1) * 128],
                            xg[:, (2 * k + h) * g:(2 * k + h + 1) * g],
                            start=(k == 0 and h == 0), stop=(k == 8 and h == 1))
                o = outp.tile([128, G], F16)
                nc.scalar.copy(o[:, :g], z[:, :g])
                nc.scalar.dma_start(out_d[dh * 128:(dh + 1) * 128, g0:g0 + g],
                                    o[:, :g])
    nc.compile()
    return nc


_NC_CACHE = None


def _get_nc():
    global _NC_CACHE
    if _NC_CACHE is None:
        _NC_CACHE = _build_nc()
    return _NC_CACHE


def _prep_core(x, seg_mask, core):
    b, r0 = core // 2, 40 * (core % 2)
    xpad = np.pad(x[b], ((1, 1), (0, 0), (0, 0)))        # [82,80,256]
    strip = xpad[r0:r0 + 42].reshape(42 * W, CH)         # [3360,256]
    sp = np.zeros((NT + 1, CH), np.float32)
    sp[1:1 + 42 * W] = strip
    A = sp[:NT].T                                        # [256, NT]
    Bs = sp[1:NT + 1].T
    xa = np.ascontiguousarray(
        np.concatenate([A[:128], A[128:]], axis=1)).astype(BF16NP)
    xb = np.ascontiguousarray(
        np.concatenate([Bs[:128], Bs[128:]], axis=1)).astype(BF16NP)

    pads = np.pad(seg_mask[b], ((1, 1), (1, 1), (0, 0)))  # [82,82,22]
    mc = seg_mask[b][r0:r0 + 40]                          # [40,80,22]
    smax = mc.max(-1, keepdims=True)
    eq = (mc == smax).astype(np.float32)
    sel = np.empty((40, 80, 9), np.float32)
    for k in range(9):
        di, dj = k // 3 - 1, k % 3 - 1
        sel[..., k] = (eq * pads[r0 + 1 + di:r0 + 41 + di,
                                 1 + dj:81 + dj]).sum(-1)
    cnt = (sel != 0).astype(np.float32).sum(-1, keepdims=True)
    selp = (sel * (9.0 / np.maximum(cnt, 1.0))).reshape(NPIX, 9)
    flat = np.concatenate(
        [selp[g0:g0 + g].T.reshape(-1) for g0, g in CHUNKS]).astype(BF16NP)
    selb = np.ascontiguousarray(np.broadcast_to(flat[None, :], (128, SELC)))
    return xa, xb, selb


def _prep_in_maps(x, seg_mask, conv_w):
    w9 = conv_w.reshape(CH, 9, DIM)
    wt = np.empty((128, WC), np.float32)
    for k in range(9):
        for h in range(2):
            for dh in range(2):
                blk = (k * 2 + h) * 2 + dh
                wt[:, blk * 128:(blk + 1) * 128] = \
                    w9[128 * h:128 * (h + 1), k, dh * 128:(dh + 1) * 128]
    wt = np.ascontiguousarray(wt).astype(BF16NP)

    in_maps = []
    for core in range(8):
        xa, xb, selb = _prep_core(x, seg_mask, core)
        in_maps.append({"xa": xa, "xb": xb, "selb": selb, "wt": wt})
    return in_maps


def kernel(x, seg_mask, conv_w):
    x = np.asarray(x, np.float32)
    seg_mask = np.asarray(seg_mask, np.float32)
    conv_w = np.asarray(conv_w, np.float32)

    in_maps = _prep_in_maps(x, seg_mask, conv_w)
    nc = _get_nc()
    res = run_bass_kernel_spmd(nc, in_maps, core_ids=list(range(8)))

    out = np.empty((B, H, W, DIM), np.float32)
    for core in range(8):
        b, r0 = core // 2, 40 * (core % 2)
        out[b, r0:r0 + 40] = res.results[core]["out"].astype(
            np.float32).T.reshape(ROWS, W, DIM)
    return out


# revision 6
# speedup vs baseline: 1.2699x; 1.0323x over previous
import sys
from contextlib import ExitStack

import numpy as np
import ml_dtypes

sys.path.insert(0, "/opt/trn_rl_repo")

try:
    import jax
    jax.config.update("jax_compilation_cache_dir", "/tmp/jax_cc_cache")
    jax.config.update("jax_persistent_cache_min_compile_time_secs", 0.0)
    jax.config.update("jax_persistent_cache_min_entry_size_bytes", 0)
except Exception:
    pass

import concourse.bass as bass  # noqa: F401
import concourse.tile as tile
from concourse import bacc, mybir
from concourse.bass_utils import run_bass_kernel_spmd

B, H, W, CH = 4, 80, 80, 256
NCLS, DIM = 22, 256
ROWS = 40                 # rows per core
NPIX = ROWS * W           # 3200 output pixels per core
NT = (ROWS + 2) * W + 2   # 3362 strip positions (halo rows + end pad)
G = 512                   # pixel chunk
CHUNKS = [(g, min(G, NPIX - g)) for g in range(0, NPIX, G)]  # 6x512 + 128
SELC = 9 * NPIX           # 28800 sel columns (chunk-major, k-major, pixel)
WC = 36 * 128             # 4608 weight cols: block (k,h,dh) of [128c x 128d]
# x piece boundaries (cols of the strip); chunk g needs cols < g0+G+163
XPB = [0, 675, 1700, 2723, NT]
F32 = mybir.dt.float32
F16 = mybir.dt.float16
BF16 = mybir.dt.bfloat16
BF16NP = ml_dtypes.bfloat16


def _build_nc():
    nc = bacc.Bacc("TRN2", target_bir_lowering=False, debug=False,
                   enable_asserts=False, num_devices=8)
    xa_d = nc.dram_tensor("xa", [128, 2 * NT], BF16, kind="ExternalInput").ap()
    xb_d = nc.dram_tensor("xb", [128, 2 * NT], BF16, kind="ExternalInput").ap()
    sel_d = nc.dram_tensor("selb", [128, SELC], BF16, kind="ExternalInput").ap()
    wt_d = nc.dram_tensor("wt", [128, WC], BF16, kind="ExternalInput").ap()
    out_d = nc.dram_tensor("out", [2 * 128, NPIX], F16, kind="ExternalOutput").ap()

    sel_off = [0]
    for _, g in CHUNKS:
        sel_off.append(sel_off[-1] + 9 * g)

    with tile.TileContext(nc) as tc, ExitStack() as ctx:
        xp = ctx.enter_context(tc.tile_pool(name="xp", bufs=1))
        xgp = ctx.enter_context(tc.tile_pool(name="xgp", bufs=3))
        outp = ctx.enter_context(tc.tile_pool(name="outp", bufs=4))
        zp = ctx.enter_context(tc.tile_pool(name="zp", bufs=4, space="PSUM"))

        xa_t = xp.tile([128, 2 * NT], BF16)
        xb_t = xp.tile([128, 2 * NT], BF16)
        wt_t = xp.tile([128, WC], BF16)
        S_t = xp.tile([128, SELC], BF16)

        # ---- all input DMAs upfront, need-ordered, on 3 parallel queues ----
        # SP queue: x strips, piece by piece, h-interleaved
        for a, b in zip(XPB[:-1], XPB[1:]):
            for t, d in ((xa_t, xa_d), (xb_t, xb_d)):
                for h in range(2):
                    nc.sync.dma_start(t[:, h * NT + a:h * NT + b],
                                      d[:, h * NT + a:h * NT + b])
        # Act queue: selb; first two chunks split per-3-taps for fast start
        sel_pieces = []
        for ci in (0, 1):
            a, b = sel_off[ci], sel_off[ci + 1]
            th = (b - a) // 3
            sel_pieces += [(a, a + th), (a + th, a + 2 * th), (a + 2 * th, b)]
        for ci in range(2, len(CHUNKS)):
            sel_pieces.append((sel_off[ci], sel_off[ci + 1]))
        for a, b in sel_pieces:
            nc.scalar.dma_start(S_t[:, a:b], sel_d[:, a:b])
        # gpsimd queue: weights in thirds (first third covers first 12 blocks)
        for a, b in ((0, 1536), (1536, 3072), (3072, WC)):
            nc.gpsimd.dma_start(wt_t[:, a:b], wt_d[:, a:b])

        # ---- main pipeline ----
        for ci, (g0, g) in enumerate(CHUNKS):
            so = sel_off[ci]
            xg = xgp.tile([128, 18 * G], BF16)
            for k in range(9):
                di, dj = k // 3, k % 3
                for h in range(2):
                    if dj == 1:
                        src = xb_t[:, h * NT + g0 + 80 * di:
                                   h * NT + g0 + 80 * di + g]
                    else:
                        src = xa_t[:, h * NT + g0 + 80 * di + dj:
                                   h * NT + g0 + 80 * di + dj + g]
                    nc.vector.tensor_mul(xg[:, (2 * k + h) * g:(2 * k + h + 1) * g],
                                         src, S_t[:, so + k * g:so + (k + 1) * g])

            for dh in range(2):
                z = zp.tile([128, G], F32)
                for k in range(9):
                    for h in range(2):
                        blk = (k * 2 + h) * 2 + dh
                        nc.tensor.matmul(
                            z[:, :g],
                            wt_t[:, blk * 128:(blk + 1) * 128],
                            xg[:, (2 * k + h) * g:(2 * k + h + 1) * g],
                            start=(k == 0 and h == 0), stop=(k == 8 and h == 1))
                o = outp.tile([128, G], F16)
                nc.scalar.copy(o[:, :g], z[:, :g])
                nc.scalar.dma_start(out_d[dh * 128:(dh + 1) * 128, g0:g0 + g],
                                    o[:, :g])
    nc.compile()
    return nc


_NC_CACHE = None


def _get_nc():
    global _NC_CACHE
    if _NC_CACHE is None:
        _NC_CACHE = _build_nc()
    return _NC_CACHE


def _prep_core(x, seg_mask, core):
    b, r0 = core // 2, 40 * (core % 2)
    xpad = np.pad(x[b], ((1, 1), (0, 0), (0, 0)))        # [82,80,256]
    strip = xpad[r0:r0 + 42].reshape(42 * W, CH)         # [3360,256]
    sp = np.zeros((NT + 1, CH), np.float32)
    sp[1:1 + 42 * W] = strip
    A = sp[:NT].T                                        # [256, NT]
    Bs = sp[1:NT + 1].T
    xa = np.ascontiguousarray(
        np.concatenate([A[:128], A[128:]], axis=1)).astype(BF16NP)
    xb = np.ascontiguousarray(
        np.concatenate([Bs[:128], Bs[128:]], axis=1)).astype(BF16NP)

    pads = np.pad(seg_mask[b], ((1, 1), (1, 1), (0, 0)))  # [82,82,22]
    mc = seg_mask[b][r0:r0 + 40]                          # [40,80,22]
    smax = mc.max(-1, keepdims=True)
    eq = (mc == smax).astype(np.float32)
    sel = np.empty((40, 80, 9), np.float32)
    for k in range(9):
        di, dj = k // 3 - 1, k % 3 - 1
        sel[..., k] = (eq * pads[r0 + 1 + di:r0 + 41 + di,
                                 1 + dj:81 + dj]).sum(-1)
    cnt = (sel != 0).astype(np.float32).sum(-1, keepdims=True)
    selp = (sel * (9.0 / np.maximum(cnt, 1.0))).reshape(NPIX, 9)
    flat = np.concatenate(
        [selp[g0:g0 + g].T.reshape(-1) for g0, g in CHUNKS]).astype(BF16NP)
    selb = np.ascontiguousarray(np.broadcast_to(flat[None, :], (128, SELC)))
    return xa, xb, selb


def _prep_in_maps(x, seg_mask, conv_w):
    w9 = conv_w.reshape(CH, 9, DIM)
    wt = np.empty((128, WC), np.float32)
    for k in range(9):
        for h in range(2):
            for dh in range(2):
                blk = (k * 2 + h) * 2 + dh
                wt[:, blk * 128:(blk + 1) * 128] = \
                    w9[128 * h:128 * (h + 1), k, dh * 128:(dh + 1) * 128]
    wt = np.ascontiguousarray(wt).astype(BF16NP)

    in_maps = []
    for core in range(8):
        xa, xb, selb = _prep_core(x, seg_mask, core)
        in_maps.append({"xa": xa, "xb": xb, "selb": selb, "wt": wt})
    return in_maps


def kernel(x, seg_mask, conv_w):
    x = np.asarray(x, np.float32)
    seg_mask = np.asarray(seg_mask, np.float32)
    conv_w = np.asarray(conv_w, np.float32)

    in_maps = _prep_in_maps(x, seg_mask, conv_w)
    nc = _get_nc()
    res = run_bass_kernel_spmd(nc, in_maps, core_ids=list(range(8)))

    out = np.empty((B, H, W, DIM), np.float32)
    for core in range(8):
        b, r0 = core // 2, 40 * (core % 2)
        out[b, r0:r0 + 40] = res.results[core]["out"].astype(
            np.float32).T.reshape(ROWS, W, DIM)
    return out
